# revision 20
# baseline (speedup 1.0000x reference)
"""DenseGCN Trainium2 kernel (8-core SPMD).

Strategy (1D node partitioning, edge-cut by dst):
- Nodes are range-sharded: core c owns nodes [c*NPC, (c+1)*NPC).
- Edges live on the core that owns their dst. Within a core, edges are
  bucketed by 128-node dst group and padded so every group has exactly
  K tiles of 128 edges (K derived from the data at build time).
- Per edge tile: dma_gather pulls node-PAIR rows (512 B) from a
  DRAM-replicated h table ([N/2, 2H] view, int16 pair index = src>>1),
  ScalarE scales the lo/hi half by ew*(1-src&1) / ew*(src&1), DVE builds
  a one-hot dst matrix C via is_equal(iota, local_dst), and two TensorE
  matmuls (lhsT=C, rhs=scaled lo/hi halves) accumulate the segment sum
  for the dst group directly in PSUM.  No scatter DMA anywhere.
- Group epilogue: PSUM agg -> transpose -> conv matmul -> +bias -> LN ->
  residual bookkeeping, all on-chip into an SBUF-resident h shard.
- Between conv layers an AllGather replicates the 2 MB h shards into the
  16 MB gather table.
"""

import math

import numpy as np

import concourse.bacc as bacc
import concourse.bass as bass
import concourse.mybir as mybir
import concourse.tile as tile
from concourse import library_config
from concourse.bass_utils import run_bass_kernel_spmd

F32 = mybir.dt.float32
I16 = mybir.dt.int16
I32 = mybir.dt.int32
AF = mybir.ActivationFunctionType
ALU = mybir.AluOpType
AX = mybir.AxisListType

NC_CORES = 8
F_IN = 128
H = 64
OUT = 32
L = 3
EPS = 1e-5
GROUP = 128  # dst nodes per segment-sum group (= PE output partitions)
P = 128

# debug bisect switches: "convs" (skip conv layers), "gather" (memset
# instead of dma_gather), "collective" (plain DMA instead of AllGather)
DEBUG_SKIP = set()


def _wrap_tile_major(v, T):
    """[T*128] -> [128, T] with v[t*128+p] at [p, t]."""
    return np.ascontiguousarray(v.reshape(T, P).T)


def _wrap_idx16(idx, E_s):
    """[E_s] int -> [128, E_s/16] int16, 16-partition wrap replicated 8x."""
    assert E_s % 16 == 0
    w16 = idx.reshape(E_s // 16, 16).T.astype(np.int16)  # [16, E_s/16]
    return np.ascontiguousarray(np.tile(w16, (8, 1)))  # [128, E_s/16]


def prep_inputs(x, edge_weight, src, dst, n_nodes, npc):
    """Host-side shard + edge bucketing. Returns per-core dicts + K."""
    ew = edge_weight.reshape(-1).astype(np.float32)
    src = src.astype(np.int64)
    dst = dst.astype(np.int64)
    ngroups = npc // GROUP
    core_of = dst // npc

    per_core = []
    counts_all = []
    for c in range(NC_CORES):
        m = core_of == c
        s_c, d_c, w_c = src[m], dst[m], ew[m]
        g = (d_c % npc) // GROUP
        order = np.argsort(g, kind="stable")
        s_c, d_c, w_c, g = s_c[order], d_c[order], w_c[order], g[order]
        cnt = np.bincount(g, minlength=ngroups)
        counts_all.append(cnt)
        per_core.append((s_c, d_c, w_c, g, cnt))

    K = int(max(math.ceil(int(c.max()) / P) for c in counts_all))
    K = max(K, 1)
    E_s = ngroups * K * P

    maps = []
    for c in range(NC_CORES):
        s_c, d_c, w_c, g, cnt = per_core[c]
        # slot index for each edge: group base + position within group
        starts = np.zeros(ngroups, dtype=np.int64)
        starts[1:] = np.cumsum(cnt)[:-1]
        within = np.arange(len(g)) - starts[g]
        slot = g * (K * P) + within
        pair = np.zeros(E_s, dtype=np.int64)
        ew_lo = np.zeros(E_s, dtype=np.float32)
        ew_hi = np.zeros(E_s, dtype=np.float32)
        ldst = np.zeros(E_s, dtype=np.float32)
        pair[slot] = s_c >> 1
        sel = (s_c & 1).astype(np.float32)
        ew_lo[slot] = w_c * (1.0 - sel)
        ew_hi[slot] = w_c * sel
        ldst[slot] = (d_c % GROUP).astype(np.float32)
        T = E_s // P
        maps.append(
            {
                "x": np.ascontiguousarray(x[c * npc : (c + 1) * npc]).astype(
                    np.float32
                ),
                "eidx": _wrap_idx16(pair, E_s),
                "ewlo": _wrap_tile_major(ew_lo, T),
                "ewhi": _wrap_tile_major(ew_hi, T),
                "ldst": _wrap_tile_major(ldst, T),
            }
        )
    return maps, K


def build_nc(n_nodes, npc, K, ln_identity, tiles_per_call):
    """Build the SPMD Bass program (same program all 8 cores)."""
    ngroups = npc // GROUP
    ntile_node = npc // P  # node tiles per core
    T = ngroups * K  # edge tiles per layer
    E_s = T * P
    # SWDGE descriptor ring carveout is 1024 descs — one dma_gather must
    # not exceed 1024 indices (HW crashes beyond that; verified on HW).
    assert tiles_per_call * P <= 1024, tiles_per_call

    nc = bacc.Bacc(None, target_bir_lowering=False)

    # ---- I/O ----
    x_d = nc.declare_dram_parameter("x", [npc, F_IN], F32, isOutput=False)
    eidx_d = nc.declare_dram_parameter("eidx", [P, E_s // 16], I16, isOutput=False)
    ewlo_d = nc.declare_dram_parameter("ewlo", [P, T], F32, isOutput=False)
    ewhi_d = nc.declare_dram_parameter("ewhi", [P, T], F32, isOutput=False)
    ldst_d = nc.declare_dram_parameter("ldst", [P, T], F32, isOutput=False)
    w1_d = nc.declare_dram_parameter("w1", [F_IN, H], F32, isOutput=False)
    b1_d = nc.declare_dram_parameter("b1r", [P, H], F32, isOutput=False)
    cw_d = [
        nc.declare_dram_parameter(f"cw{i}", [H, H], F32, isOutput=False)
        for i in range(L)
    ]
    cb_d = [
        nc.declare_dram_parameter(f"cb{i}r", [P, H], F32, isOutput=False)
        for i in range(L)
    ]
    w3_d = nc.declare_dram_parameter("w3", [H, H], F32, isOutput=False)
    b3_d = nc.declare_dram_parameter("b3r", [P, H], F32, isOutput=False)
    w4_d = nc.declare_dram_parameter("w4", [H, OUT], F32, isOutput=False)
    b4_d = nc.declare_dram_parameter("b4r", [P, OUT], F32, isOutput=False)
    iota_d = nc.declare_dram_parameter("iotaf", [P, P], F32, isOutput=False)
    ident_d = nc.declare_dram_parameter("ident", [P, P], F32, isOutput=False)
    ln_d = {}
    if not ln_identity:
        ln_d["ln1g"] = nc.declare_dram_parameter("ln1g", [P, F_IN], F32, False)
        ln_d["ln1b"] = nc.declare_dram_parameter("ln1b", [P, F_IN], F32, False)
        ln_d["lng"] = nc.declare_dram_parameter("lng", [P, H], F32, False)
        ln_d["lnb"] = nc.declare_dram_parameter("lnb", [P, H], F32, False)
        ln_d["ln2g"] = nc.declare_dram_parameter("ln2g", [P, H], F32, False)
        ln_d["ln2b"] = nc.declare_dram_parameter("ln2b", [P, H], F32, False)
    out_d = nc.declare_dram_parameter("out", [npc, OUT], F32, isOutput=True)

    # ---- internal DRAM ----
    h_bounce = nc.dram_tensor("h_bounce", [npc, H], F32)
    # full-h gather table, viewed as node pairs: [N/2, 2H]
    h_full = nc.dram_tensor("h_full", [n_nodes // 2, 2 * H], F32, addr_space="Shared")

    groups_all = [list(range(NC_CORES))]

    with tile.TileContext(nc) as tc:
        with (
            tc.tile_pool(name="const", bufs=1) as cpool,
            tc.tile_pool(name="gpool", bufs=2) as gpool,
            tc.tile_pool(name="edge", bufs=4) as epool,
            tc.tile_pool(name="work", bufs=3) as wpool,
            tc.tile_pool(name="stat", bufs=4) as spool,
            tc.tile_pool(name="psA", bufs=2, space="PSUM") as psA,
            tc.tile_pool(name="psB", bufs=2, space="PSUM") as psB,
            tc.tile_pool(name="psC", bufs=2, space="PSUM") as psC,
        ):
            # ---------- persistent constants ----------
            nc.gpsimd.load_library(library_config.mlp)
            iota_f = cpool.tile([P, P], F32)
            nc.sync.dma_start(out=iota_f[:], in_=iota_d[:, :])
            ident = cpool.tile([P, P], F32)
            nc.sync.dma_start(out=ident[:], in_=ident_d[:, :])

            eidx_s = cpool.tile([P, E_s // 16], I16)
            nc.sync.dma_start(out=eidx_s[:], in_=eidx_d[:, :])
            ewlo_s = cpool.tile([P, T], F32)
            nc.sync.dma_start(out=ewlo_s[:], in_=ewlo_d[:, :])
            ewhi_s = cpool.tile([P, T], F32)
            nc.sync.dma_start(out=ewhi_s[:], in_=ewhi_d[:, :])
            ldst_s = cpool.tile([P, T], F32)
            nc.sync.dma_start(out=ldst_s[:], in_=ldst_d[:, :])

            w1_s = cpool.tile([F_IN, H], F32)
            nc.sync.dma_start(out=w1_s[:], in_=w1_d[:, :])
            b1_s = cpool.tile([P, H], F32)
            nc.sync.dma_start(out=b1_s[:], in_=b1_d[:, :])
            cw_s, cb_s = [], []
            for i in range(L):
                w = cpool.tile([H, H], F32, tag=f"cw{i}")
                nc.sync.dma_start(out=w[:], in_=cw_d[i][:, :])
                cw_s.append(w)
                b = cpool.tile([P, H], F32, tag=f"cb{i}")
                nc.sync.dma_start(out=b[:], in_=cb_d[i][:, :])
                cb_s.append(b)
            w3_s = cpool.tile([H, H], F32, tag="w3")
            nc.sync.dma_start(out=w3_s[:], in_=w3_d[:, :])
            b3_s = cpool.tile([P, H], F32, tag="b3")
            nc.sync.dma_start(out=b3_s[:], in_=b3_d[:, :])
            w4_s = cpool.tile([H, OUT], F32, tag="w4")
            nc.sync.dma_start(out=w4_s[:], in_=w4_d[:, :])
            b4_s = cpool.tile([P, OUT], F32, tag="b4")
            nc.sync.dma_start(out=b4_s[:], in_=b4_d[:, :])
            ln_s = {}
            for k in ln_d:
                f = F_IN if k.startswith("ln1") else H
                t_ = cpool.tile([P, f], F32, tag=k)
                nc.sync.dma_start(out=t_[:], in_=ln_d[k][:, :])
                ln_s[k] = t_

            h_stage = cpool.tile([P, ntile_node * H], F32, tag="hstage")
            res_sum = cpool.tile([P, ntile_node * H], F32, tag="ressum")
            out_stage = cpool.tile([P, ntile_node * OUT], F32, tag="ostage")

            # ---------- helpers ----------
            def layer_norm(dst_ap, src_ap, f, gkey=None, bkey=None):
                """dst = LN(src) along free axis of width f. src/dst [P?, f]."""
                parts = src_ap.shape[0]
                ssum = spool.tile([P, 1], F32, tag="lnsum")
                nc.vector.tensor_reduce(
                    out=ssum[:parts], in_=src_ap, axis=AX.X, op=ALU.add
                )
                mean = spool.tile([P, 1], F32, tag="lnmean")
                nc.vector.tensor_scalar_mul(mean[:parts], ssum[:parts], 1.0 / f)
                xc = wpool.tile([P, f], F32, tag=f"lnxc{f}")
                nc.vector.tensor_scalar(
                    out=xc[:parts],
                    in0=src_ap,
                    scalar1=mean[:parts, 0:1],
                    scalar2=None,
                    op0=ALU.subtract,
                )
                sq = wpool.tile([P, f], F32, tag=f"lnsq{f}")
                nc.vector.tensor_tensor(
                    out=sq[:parts], in0=xc[:parts], in1=xc[:parts], op=ALU.mult
                )
                vsum = spool.tile([P, 1], F32, tag="lnvar")
                nc.vector.tensor_reduce(
                    out=vsum[:parts], in_=sq[:parts], axis=AX.X, op=ALU.add
                )
                veps = spool.tile([P, 1], F32, tag="lnveps")
                nc.vector.tensor_scalar(
                    out=veps[:parts],
                    in0=vsum[:parts],
                    scalar1=1.0 / f,
                    scalar2=EPS,
                    op0=ALU.mult,
                    op1=ALU.add,
                )
                std = spool.tile([P, 1], F32, tag="lnstd")
                nc.scalar.sqrt(std[:parts], veps[:parts])
                rstd = spool.tile([P, 1], F32, tag="lnrstd")
                nc.vector.reciprocal(rstd[:parts], std[:parts])
                if gkey is None:
                    nc.vector.tensor_scalar(
                        out=dst_ap,
                        in0=xc[:parts],
                        scalar1=rstd[:parts, 0:1],
                        scalar2=None,
                        op0=ALU.mult,
                    )
                else:
                    nrm = wpool.tile([P, f], F32, tag=f"lnnrm{f}")
                    nc.vector.tensor_scalar(
                        out=nrm[:parts],
                        in0=xc[:parts],
                        scalar1=rstd[:parts, 0:1],
                        scalar2=None,
                        op0=ALU.mult,
                    )
                    tmp = wpool.tile([P, f], F32, tag=f"lnaf{f}")
                    nc.vector.tensor_tensor(
                        out=tmp[:parts],
                        in0=nrm[:parts],
                        in1=ln_s[gkey][:parts],
                        op=ALU.mult,
                    )
                    nc.vector.tensor_tensor(
                        out=dst_ap,
                        in0=tmp[:parts],
                        in1=ln_s[bkey][:parts],
                        op=ALU.add,
                    )

            def elu(dst_ap, src_ap, f):
                """dst = ELU(src) = (max(x,0)-1) + exp(min(x,0))."""
                parts = src_ap.shape[0]
                r1 = wpool.tile([P, f], F32, tag=f"elur{f}")
                nc.vector.tensor_scalar(
                    out=r1[:parts],
                    in0=src_ap,
                    scalar1=0.0,
                    scalar2=1.0,
                    op0=ALU.max,
                    op1=ALU.subtract,
                )
                mn = wpool.tile([P, f], F32, tag=f"elum{f}")
                nc.vector.tensor_scalar(
                    out=mn[:parts],
                    in0=src_ap,
                    scalar1=0.0,
                    scalar2=None,
                    op0=ALU.min,
                )
                ex = wpool.tile([P, f], F32, tag=f"elue{f}")
                nc.scalar.activation(ex[:parts], mn[:parts], AF.Exp)
                nc.vector.tensor_tensor(
                    out=dst_ap, in0=r1[:parts], in1=ex[:parts], op=ALU.add
                )

            # ---------- fc_first ----------
            for t in range(ntile_node):
                xt = wpool.tile([P, F_IN], F32, tag="xt")
                nc.sync.dma_start(out=xt[:], in_=x_d[t * P : (t + 1) * P, :])
                lnx = wpool.tile([P, F_IN], F32, tag="lnx")
                if ln_identity:
                    layer_norm(lnx[:], xt[:], F_IN)
                else:
                    layer_norm(lnx[:], xt[:], F_IN, "ln1g", "ln1b")
                xT_ps = psB.tile([P, P], F32, tag="trps")
                nc.tensor.transpose(out=xT_ps[:], in_=lnx[:], identity=ident[:])
                xT = wpool.tile([P, P], F32, tag="xT")
                nc.scalar.activation(xT[:], xT_ps[:], AF.Copy)
                h_ps = psC.tile([P, H], F32, tag="linps")
                nc.tensor.matmul(
                    out=h_ps[:], lhsT=xT[:], rhs=w1_s[:], start=True, stop=True
                )
                hb = wpool.tile([P, H], F32, tag="hb")
                nc.vector.tensor_tensor(
                    out=hb[:], in0=h_ps[:], in1=b1_s[:], op=ALU.add
                )
                he = wpool.tile([P, H], F32, tag="he")
                elu(he[:], hb[:], H)
                sl = slice(t * H, (t + 1) * H)
                if ln_identity:
                    layer_norm(h_stage[:, sl], he[:], H)
                else:
                    layer_norm(h_stage[:, sl], he[:], H, "lng", "lnb")
                nc.scalar.activation(res_sum[:, sl], h_stage[:, sl], AF.Copy)

            # DRAM view of h_bounce matching h_stage layout:
            # h_bounce[n, f], n = t*128 + p  ->  [p, t, f]
            hb_v = h_bounce[:, :].rearrange("(t p) f -> p t f", p=P)

            # ---------- conv layers ----------
            for li in range(L if "convs" not in DEBUG_SKIP else 0):
                # replicate h: shard -> bounce -> AllGather -> h_full
                nc.sync.dma_start(out=hb_v, in_=h_stage[:].rearrange(
                    "p (t f) -> p t f", f=H))
                if "collective" in DEBUG_SKIP:
                    # hang-test only: copy own shard into the table slot 0
                    nc.sync.dma_start(
                        out=h_full[: npc // 2, :], in_=h_bounce[:, :]
                    )
                else:
                    nc.gpsimd.collective_compute(
                        "AllGather",
                        ALU.bypass,
                        replica_groups=groups_all,
                        ins=[h_bounce[:, :]],
                        outs=[h_full[:, :]],
                    )
                for g in range(ngroups):
                    agg_ps = psA.tile([P, H], F32, tag="aggps")
                    for k in range(K):
                        t = g * K + k
                        tt = t % tiles_per_call
                        if tt == 0:
                            c0 = t  # first tile of this call
                            n_t = min(tiles_per_call, T - c0)
                            n_e = n_t * P
                            gbuf = gpool.tile([P, n_t, P], F32, tag="gbuf")
                            if "gather" in DEBUG_SKIP:
                                nc.vector.memset(gbuf[:], 0.0)
                            else:
                                nc.gpsimd.dma_gather(
                                    out_ap=gbuf[:],
                                    in_ap=h_full[:, :],
                                    idxs_ap=eidx_s[
                                        :,
                                        c0 * (P // 16) : c0 * (P // 16)
                                        + (n_e // 16),
                                    ],
                                    num_idxs=n_e,
                                    num_idxs_reg=n_e,
                                    elem_size=2 * H,
                                )
                        t1 = epool.tile([P, H], F32, tag="t1")
                        nc.scalar.activation(
                            t1[:],
                            gbuf[:, tt, 0:H],
                            AF.Copy,
                            scale=ewlo_s[:, t : t + 1],
                        )
                        t2 = epool.tile([P, H], F32, tag="t2")
                        nc.scalar.activation(
                            t2[:],
                            gbuf[:, tt, H : 2 * H],
                            AF.Copy,
                            scale=ewhi_s[:, t : t + 1],
                        )
                        cm = epool.tile([P, P], F32, tag="cm")
                        nc.vector.tensor_scalar(
                            out=cm[:],
                            in0=iota_f[:],
                            scalar1=ldst_s[:, t : t + 1],
                            scalar2=None,
                            op0=ALU.is_equal,
                        )
                        nc.tensor.matmul(
                            out=agg_ps[:],
                            lhsT=cm[:],
                            rhs=t1[:],
                            start=(k == 0),
                            stop=False,
                        )
                        nc.tensor.matmul(
                            out=agg_ps[:],
                            lhsT=cm[:],
                            rhs=t2[:],
                            start=False,
                            stop=(k == K - 1),
                        )
                    # --- group epilogue ---
                    agg_s = wpool.tile([P, H], F32, tag="aggs")
                    nc.scalar.activation(agg_s[:], agg_ps[:], AF.Copy)
                    tr_ps = psB.tile([H, P], F32, tag="trps")
                    nc.tensor.transpose(
                        out=tr_ps[:], in_=agg_s[:], identity=ident[:]
                    )
                    aggT = wpool.tile([H, P], F32, tag="aggT")
                    nc.scalar.activation(aggT[:], tr_ps[:], AF.Copy)
                    lin_ps = psC.tile([P, H], F32, tag="linps")
                    nc.tensor.matmul(
                        out=lin_ps[:],
                        lhsT=aggT[:],
                        rhs=cw_s[li][:],
                        start=True,
                        stop=True,
                    )
                    hbt = wpool.tile([P, H], F32, tag="hb")
                    nc.vector.tensor_tensor(
                        out=hbt[:], in0=lin_ps[:], in1=cb_s[li][:], op=ALU.add
                    )
                    hn = wpool.tile([P, H], F32, tag="hn")
                    if ln_identity:
                        layer_norm(hn[:], hbt[:], H)
                    else:
                        layer_norm(hn[:], hbt[:], H, "lng", "lnb")
                    gsl = slice(g * H, (g + 1) * H)
                    nc.vector.tensor_tensor(
                        out=h_stage[:, gsl],
                        in0=hn[:],
                        in1=res_sum[:, gsl],
                        op=ALU.add,
                    )
                    nc.vector.tensor_tensor(
                        out=res_sum[:, gsl],
                        in0=res_sum[:, gsl],
                        in1=hn[:],
                        op=ALU.add,
                    )

            # ---------- fc_final ----------
            for t in range(ntile_node):
                sl = slice(t * H, (t + 1) * H)
                lnh = wpool.tile([P, H], F32, tag="lnh")
                if ln_identity:
                    layer_norm(lnh[:], h_stage[:, sl], H)
                else:
                    layer_norm(lnh[:], h_stage[:, sl], H, "ln2g", "ln2b")
                tr_ps = psB.tile([H, P], F32, tag="trps")
                nc.tensor.transpose(out=tr_ps[:], in_=lnh[:], identity=ident[:])
                lnhT = wpool.tile([H, P], F32, tag="aggT")
                nc.scalar.activation(lnhT[:], tr_ps[:], AF.Copy)
                z_ps = psC.tile([P, H], F32, tag="linps")
                nc.tensor.matmul(
                    out=z_ps[:], lhsT=lnhT[:], rhs=w3_s[:], start=True, stop=True
                )
                zb = wpool.tile([P, H], F32, tag="hb")
                nc.vector.tensor_tensor(
                    out=zb[:], in0=z_ps[:], in1=b3_s[:], op=ALU.add
                )
                ze = wpool.tile([P, H], F32, tag="he")
                elu(ze[:], zb[:], H)
                tr2_ps = psB.tile([H, P], F32, tag="trps")
                nc.tensor.transpose(out=tr2_ps[:], in_=ze[:], identity=ident[:])
                zT = wpool.tile([H, P], F32, tag="aggT")
                nc.scalar.activation(zT[:], tr2_ps[:], AF.Copy)
                o_ps = psC.tile([P, OUT], F32, tag="ops")
                nc.tensor.matmul(
                    out=o_ps[:], lhsT=zT[:], rhs=w4_s[:], start=True, stop=True
                )
                osl = slice(t * OUT, (t + 1) * OUT)
                nc.vector.tensor_tensor(
                    out=out_stage[:, osl], in0=o_ps[:], in1=b4_s[:], op=ALU.add
                )

            out_v = out_d[:, :].rearrange("(t p) f -> p t f", p=P)
            nc.sync.dma_start(
                out=out_v,
                in_=out_stage[:].rearrange("p (t f) -> p t f", f=OUT),
            )

    nc.compile()
    return nc


def _replicate(v, parts=P):
    return np.ascontiguousarray(np.tile(np.asarray(v, np.float32)[None, :], (parts, 1)))


def kernel(
    x,
    edge_weight,
    src,
    dst,
    ln1_g,
    ln1_b,
    w1,
    b1,
    ln_g,
    ln_b,
    conv_w,
    conv_b,
    ln2_g,
    ln2_b,
    w3,
    b3,
    w4,
    b4,
    _n_cores=NC_CORES,
    _tiles_per_call=None,
    _trace=False,
):
    x = np.asarray(x, np.float32)
    n_nodes = x.shape[0]
    npc = n_nodes // NC_CORES

    ln_identity = (
        np.all(ln1_g == 1) and np.all(ln1_b == 0)
        and np.all(ln_g == 1) and np.all(ln_b == 0)
        and np.all(ln2_g == 1) and np.all(ln2_b == 0)
    )

    maps, K = prep_inputs(x, np.asarray(edge_weight), np.asarray(src),
                          np.asarray(dst), n_nodes, npc)

    tiles_per_call = _tiles_per_call
    if tiles_per_call == "K":
        tiles_per_call = K
    elif tiles_per_call is None:
        tiles_per_call = 8  # 1024 idxs = SWDGE ring capacity

    weights = {
        "iotaf": np.tile(np.arange(P, dtype=np.float32)[None, :], (P, 1)),
        "ident": np.eye(P, dtype=np.float32),
        "w1": np.asarray(w1, np.float32),
        "b1r": _replicate(b1),
        "w3": np.asarray(w3, np.float32),
        "b3r": _replicate(b3),
        "w4": np.asarray(w4, np.float32),
        "b4r": _replicate(b4),
    }
    for i in range(L):
        weights[f"cw{i}"] = np.asarray(conv_w[i], np.float32)
        weights[f"cb{i}r"] = _replicate(conv_b[i])
    if not ln_identity:
        weights["ln1g"] = _replicate(ln1_g)
        weights["ln1b"] = _replicate(ln1_b)
        weights["lng"] = _replicate(ln_g)
        weights["lnb"] = _replicate(ln_b)
        weights["ln2g"] = _replicate(ln2_g)
        weights["ln2b"] = _replicate(ln2_b)

    in_maps = [{**m, **weights} for m in maps]

    nc = build_nc(n_nodes, npc, K, ln_identity, tiles_per_call)
    res = run_bass_kernel_spmd(
        nc, in_maps, core_ids=list(range(NC_CORES)), trace=_trace
    )
    global LAST_RESULTS
    LAST_RESULTS = res
    return np.concatenate([r["out"] for r in res.results], axis=0)


LAST_RESULTS = None
